# revision 3
# baseline (speedup 1.0000x reference)
"""Involution2d (B=8, C=256, H=W=56, K=7, G=16, reduction=4) on 8 TRN2 NeuronCores.

Sharding: spatial over H (7 output rows per core, 3-row halos), full batch
on-chip.  Involution partition layout = (group g, batch b) = 128 partitions:
per-pixel kernel maps broadcast across the 16 channels of their group via a
free-dim step-0 AP, tap shifts are free-dim offsets into padded x rows.

This environment charges ~1us per non-contiguous run in DMA/compute APs and
~80us latency per dependency-chained op, so v2 minimizes short runs and
dependency depth:
  - kernel-generation matmul chunks are tap-major (112 cols = 16 groups x 7
    taps of one kh row); the partition rearrange (g,k),(b,p) -> (g,b),(k,p)
    is a contiguous DRAM write plus per-group strided reads split across
    both HWDGE queues.
  - involution taps multiply over the full padded width (runs of 448) and
    accumulate via a depth-3 tree per kh row, fp32 master accumulator.
  - output is stored as one flat [128, 7168] DMA; the host unpacks.
"""

import os
import sys

import numpy as np

for _p in ("/opt/trn_rl_repo",):
    if os.path.isdir(_p) and _p not in sys.path:
        sys.path.insert(0, _p)

import concourse.bacc as bacc
import concourse.mybir as mybir
from concourse.ap import AP
from concourse.tile import TileContext
from concourse.bass_utils import run_bass_kernel_spmd

# Problem constants (hardcoded per the task contract).
B, C, H, W = 8, 256, 56, 56
G, K, PAD = 16, 7, 3
CPG = C // G            # 16 channels per group
KK = K * K              # 49 taps
CR = 64                 # reduced channels
NCORES = 8
HS = H // NCORES        # 7 rows per core
HALO = PAD
HP = HS + 2 * HALO      # 13 padded rows
LPAD = 4                # left W-pad (even -> bf16 4B alignment)
WP = 64                 # padded row width: 4 + 56 + 4
NPIX = HS * WP          # 448 padded pixels per sample slab
NALLP = B * NPIX        # 3584 matmul moving dim
CROW = HP * WP          # 832 x elems per (c') row
XFLAT = CPG * CROW      # 13312 flat x elems per partition
XPAD = 14336            # x tile free size (lead pad + data + slack)
XOFF = 8                # x data offset inside the tile
NF = CPG * NPIX         # 7168 involution elems per partition

F32 = mybir.dt.float32
BF16 = mybir.dt.bfloat16

MCHUNK = G * K          # 112 ker rows per chunk = one kh row, all groups
NCHUNKS = K             # 7 chunks
NHALF = NALLP // 2      # 1792


def _build(reps=1):
    nc = bacc.Bacc(trn_type="TRN2")

    xs = nc.dram_tensor("xs", [B, C, HP, WP], F32, kind="ExternalInput").ap()
    xsmm = nc.dram_tensor("xsmm", [C, NALLP], F32, kind="ExternalInput").ap()
    w1t = nc.dram_tensor("w1t", [C, CR], F32, kind="ExternalInput").ap()
    b1 = nc.dram_tensor("b1", [CR, 1], F32, kind="ExternalInput").ap()
    # tap-major permuted: column j*112 + g*7 + kk = w_span row (g*49+j*7+kk)
    w2t = nc.dram_tensor("w2t", [CR, G * KK], F32, kind="ExternalInput").ap()
    b2 = nc.dram_tensor("b2", [MCHUNK, NCHUNKS], F32, kind="ExternalInput").ap()
    out = nc.dram_tensor("out", [128, NF], F32, kind="ExternalOutput").ap()
    kscratch = nc.dram_tensor(
        "kscratch", [NCHUNKS, MCHUNK, NALLP], BF16
    ).ap()

    with TileContext(nc) as tc:
        with (
            tc.tile_pool(name="const", bufs=1) as cpool,
            tc.tile_pool(name="xp", bufs=1) as xpool,
            tc.tile_pool(name="work", bufs=1) as wpool,
            tc.tile_pool(name="stage", bufs=2) as spool,
            tc.tile_pool(name="pp", bufs=4) as prodpool,
            tc.tile_pool(name="psum", bufs=2, space="PSUM") as ppool,
        ):
            # ---------------- weights / biases ----------------
            lhsT1 = []
            for i in range(2):
                t = cpool.tile([128, CR], BF16, tag=f"w1_{i}", name=f"w1_{i}")
                nc.gpsimd.dma_start(out=t[:, :], in_=w1t[i * 128:(i + 1) * 128, :])
                lhsT1.append(t)
            w2all = cpool.tile([CR, G * KK], BF16, tag="w2", name="w2all")
            nc.gpsimd.dma_start(out=w2all[:, :], in_=w2t[:, :])
            lhsT2 = [w2all[:, j * MCHUNK:(j + 1) * MCHUNK] for j in range(NCHUNKS)]
            b2all = cpool.tile([MCHUNK, NCHUNKS], F32, tag="b2", name="b2all")
            nc.sync.dma_start(out=b2all[:, :], in_=b2[:, :])
            b2t = [b2all[:, j:j + 1] for j in range(NCHUNKS)]
            b1t = cpool.tile([CR, 1], F32, tag="b1", name="b1")
            nc.sync.dma_start(out=b1t[:, :], in_=b1[:, :])

            # ---------------- x loads ----------------
            x_even = xpool.tile([128, XPAD], BF16, tag="xe", name="x_even")
            xs_g = xs.rearrange("b (g c) h w -> g b (c h w)", g=G)
            nc.vector.memset(x_even[:, :], 0.0)
            nc.gpsimd.dma_start(out=x_even[:, XOFF:XOFF + XFLAT], in_=xs_g)

            xmm = []
            for i in range(2):
                t = spool.tile([128, NALLP], BF16, tag="kst", bufs=3,
                               name=f"xmm_{i}")
                nc.gpsimd.dma_start(
                    out=t[:, :], in_=xsmm[i * 128:(i + 1) * 128, :]
                )
                xmm.append(t)

            z_sb = wpool.tile([CR, NALLP], BF16, tag="z", name="z_sb")
            acc = wpool.tile([128, NF], F32, tag="acc", name="acc")
            red = wpool.tile([128, NF], F32, tag="red", name="red")

            def nsplits(lo, hi):
                r = []
                n0 = lo
                while n0 < hi:
                    r.append((n0, min(hi, n0 + 512)))
                    n0 += 512
                return r

            with tc.For_i(0, reps, name="repl"):
                # ---------------- z = w_reduce @ x ----------------
                psum_z = ppool.tile(
                    [CR, NALLP], F32, tag="ps", bufs=1, name="psz"
                )
                for s in range(7):
                    a, b_ = s * 512, (s + 1) * 512
                    for i in range(2):
                        nc.tensor.matmul(
                            out=psum_z[:, a:b_],
                            lhsT=lhsT1[i][:, :],
                            rhs=xmm[i][:, a:b_],
                            start=(i == 0),
                            stop=(i == 1),
                        )
                nc.scalar.add(z_sb[:, :], psum_z[:, :], b1t[:, 0:1])

                # ---------------- ker chunks (one kh row each) -------------
                ktaps = []
                for j in range(NCHUNKS):
                    kst = spool.tile(
                        [MCHUNK, NALLP], BF16, tag="kst", bufs=3,
                        name=f"kst_{j}"
                    )
                    psum_k = ppool.tile(
                        [MCHUNK, NALLP], F32, tag="ps", bufs=1,
                        name=f"psk_{j}",
                    )
                    for s in range(7):
                        a, b_ = s * 512, (s + 1) * 512
                        nc.tensor.matmul(
                            out=psum_k[:, a:b_],
                            lhsT=lhsT2[j],
                            rhs=z_sb[:, a:b_],
                            start=True,
                            stop=True,
                        )
                    nc.scalar.add(kst[:, :], psum_k[:, :], b2t[j])
                    # contiguous spill, then per-group strided reads that
                    # land ker in [(g,b), (kk,p)] partition layout
                    weng = nc.sync if j % 2 == 0 else nc.scalar
                    weng.dma_start(out=kscratch[j, :, :], in_=kst[:, :])
                    ktap = spool.tile(
                        [128, K, NPIX], BF16, tag="ktap", bufs=3,
                        name=f"ktap_{j}"
                    )
                    for g in range(G):
                        reng = nc.sync if g % 2 == 0 else nc.scalar
                        reng.dma_start(
                            out=ktap[g * B:(g + 1) * B, :, :],
                            in_=kscratch[j, g * K:(g + 1) * K].rearrange(
                                "kk (b p) -> b kk p", b=B
                            ),
                        )
                    ktaps.append(ktap)

                # ---------------- involution on DVE ----------------
                # per kh: two overlapping-AP muls [8c,448p,7kw] + fp32
                # innermost-axis reduces, then one accumulate add
                CCH = 8
                with nc.allow_low_precision("involution bf16 products"):
                    for kh in range(K):
                        ktap = ktaps[kh]
                        dst = acc if kh == 0 else red
                        kt0 = ktap[:, 0:1, 0:1]
                        x0 = x_even[:, 0:1]
                        for v in range(CPG // CCH):
                            c0 = v * CCH
                            base = XOFF + c0 * CROW + kh * WP - PAD
                            xin = AP(
                                x0.tensor, base,
                                [list(x0.ap[0]), [CROW, CCH], [1, NPIX],
                                 [1, K]],
                            )
                            kin = AP(
                                kt0.tensor, kt0.offset,
                                [list(kt0.ap[0]), [0, CCH], [1, NPIX],
                                 [NPIX, K]],
                            )
                            m = prodpool.tile(
                                [128, CCH, NPIX, K], BF16, tag="m", bufs=1,
                                name="m",
                            )
                            nc.vector.tensor_mul(m[:, :, :, :], xin, kin)
                            nc.vector.tensor_reduce(
                                out=dst[
                                    :, c0 * NPIX:(c0 + CCH) * NPIX
                                ].rearrange("q (c p) -> q c p", c=CCH),
                                in_=m[:, :, :, :],
                                axis=mybir.AxisListType.X,
                                op=mybir.AluOpType.add,
                            )
                        if kh > 0:
                            nc.vector.tensor_add(
                                acc[:, :], acc[:, :], red[:, :]
                            )

                # ---------------- store ----------------
                nc.scalar.dma_start(out=out, in_=acc[:, :])

    return nc


_CACHE = {}


def _get_program(reps=1):
    if reps not in _CACHE:
        nc = _build(reps)
        nc.compile()
        _CACHE[reps] = nc
    return _CACHE[reps]


def _make_inputs(x, w_reduce, b_reduce, w_span, b_span):
    x = np.ascontiguousarray(np.asarray(x, dtype=np.float32))
    w1t = np.ascontiguousarray(np.asarray(w_reduce, np.float32).T)
    b1 = np.ascontiguousarray(np.asarray(b_reduce, np.float32).reshape(-1, 1))
    # permute w_span rows tap-major: chunk j gets (g, kk) -> row g*49+j*7+kk
    w_span = np.asarray(w_span, np.float32)
    b_span = np.asarray(b_span, np.float32)
    perm = np.empty(G * KK, np.int64)
    idx = 0
    for j in range(NCHUNKS):
        for g in range(G):
            for kk in range(K):
                perm[idx] = g * KK + j * K + kk
                idx += 1
    w2t = np.ascontiguousarray(w_span[perm].T)
    b2 = np.ascontiguousarray(b_span[perm].reshape(NCHUNKS, MCHUNK).T)
    in_maps = []
    for i in range(NCORES):
        h0 = i * HS - HALO
        sl = np.zeros((B, C, HP, WP), np.float32)
        s0, s1 = max(0, h0), min(H, h0 + HP)
        sl[:, :, s0 - h0:s1 - h0, LPAD:LPAD + W] = x[:, :, s0:s1, :]
        xsmm = np.ascontiguousarray(
            sl[:, :, HALO:HALO + HS, :].transpose(1, 0, 2, 3).reshape(C, NALLP)
        )
        in_maps.append({"xs": sl, "xsmm": xsmm, "w1t": w1t, "b1": b1,
                        "w2t": w2t, "b2": b2})
    return in_maps


def _unpack_out(arr):
    """[128, NF] fp32 -> [B, C, HS, W]"""
    a = arr.reshape(G, B, CPG, HS, WP)[:, :, :, :, LPAD:LPAD + W]
    return np.ascontiguousarray(a.transpose(1, 0, 2, 3, 4)).reshape(B, C, HS, W)


def kernel_with_results(x, w_reduce, b_reduce, w_span, b_span, trace=False, reps=1):
    in_maps = _make_inputs(x, w_reduce, b_reduce, w_span, b_span)
    nc = _get_program(reps)
    res = run_bass_kernel_spmd(nc, in_maps, list(range(NCORES)), trace=trace)
    full = np.concatenate(
        [_unpack_out(res.results[i]["out"]) for i in range(NCORES)], axis=2
    ).astype(np.float32)
    return full, res


def kernel(x, w_reduce, b_reduce, w_span, b_span):
    full, _ = kernel_with_results(x, w_reduce, b_reduce, w_span, b_span)
    return full

